# revision 1
# baseline (speedup 1.0000x reference)
"""Causal self-attention with RoPE on 8 Trainium2 NeuronCores.

Sharding: tensor-parallel over heads (2 heads/core) through QKV projection,
RoPE and attention; AllToAll reshards attention output from head-split to
token-split; out-projection is token-parallel with full out_w per core.

Schedule: per-batch interleave so the resharding collectives overlap compute:
  QKV(b0) -> attn(b0,h0),attn(b0,h1) -> CC(b0) ->
  QKV(b1) -> attn(b1,h0) -> CC(b1,h0) -> attn(b1,h1) -> CC(b1,h1) ->
  out-proj pass1 (b0 tokens, full contraction)
  out-proj pass2a (b1 tokens, h0-half contraction) + pass2b (h1-half).
Each core ends up owning tokens [c*256,(c+1)*256) of each batch element.

Layouts (per core, f = feature, t = token, d = contraction):
  xT   [D, NT]    input transposed (d on partitions) - rhs/lhsT for projections
  q/k  [128, T]   per-(head,batch), head-dim on partitions: proj out [f, t]
  v    [T, 256]   token-major: proj out [t, f]
  sT   [j, r]     scores transposed: lhsT=kT-tile, rhs=qT-block
  pT   [j, r]     exp(scores*scale) bf16
  oT   [dv, r]    PV: lhsT=v-tile [j, dv], rhs=pT [j, r]
  denom[1, r]     ones-matmul over fp32-accumulated pT (PE PSUM accumulation)
  recipB          K=1 PE matmul broadcasts 1/denom to 128 partitions (PSUM)
  out  [t, e]     out-proj: lhsT=zg-tile [dv, t], rhs=ow [dv, e]
"""
import math
import numpy as np
import ml_dtypes

import concourse.bass as bass
import concourse.mybir as mybir
import concourse.tile as tile
from concourse import bacc
from concourse.bass_utils import run_bass_kernel_spmd

F32 = mybir.dt.float32
BF16 = mybir.dt.bfloat16
AF = mybir.ActivationFunctionType
ALU = mybir.AluOpType

N_CORES = 8


def legalize_waits(nc, max_waits=1):
    """This walrus build only encodes one sync-wait per TPB instruction.
    Move extra waits emitted by Tile onto same-engine NoOps inserted
    immediately before the instruction."""
    n_split = 0
    for fn in nc.m.functions:
        for bb in fn.blocks:
            new_insts = []
            for inst in bb.instructions:
                si = getattr(inst, "sync_info", None)
                waits = list(si.on_wait) if si is not None and si.on_wait else []
                if len(waits) > max_waits and type(inst).__name__ != "InstNoOp":
                    extra, keep = waits[:-max_waits], waits[-max_waits:]
                    for k, w in enumerate(extra):
                        nop = mybir.InstNoOp(
                            name=f"{inst.name}_waitnop{k}",
                            engine=inst.engine,
                            ins=[],
                            outs=[],
                            sync_info=mybir.SyncInfo(on_wait=[w], on_update=[]),
                        )
                        nc.register_instruction(nop)
                        new_insts.append(nop)
                    inst.sync_info = mybir.SyncInfo(
                        on_wait=keep, on_update=list(si.on_update)
                    )
                    n_split += 1
                new_insts.append(inst)
            bb.instructions = new_insts
    return n_split


def build_nc(B=2, T=2048, D=2048, H=16, fake_cc=False, n_loop=1, dummy_io=False):
    HD = D // H                  # 128, head dim
    NT = B * T                   # total tokens
    HPC = H // N_CORES           # heads per core (2)
    DC = HPC * HD                # head channels per core (256)
    KT = D // 128                # contraction tiles for projections (16)
    RB = T // 512                # 512-token row blocks per batch element (4)
    BLK = T // 512               # 512-token qkv blocks per batch element (4)
    SB = T // N_CORES            # tokens per core per batch after AllToAll (256)
    EB = D // 512                # 512-wide out-feature blocks (4)
    SCALE = 1.0 / math.sqrt(HD)

    nc = bacc.Bacc("TRN2", target_bir_lowering=False, debug=False, num_devices=N_CORES)
    ik = {"kind": "ExternalInput"} if not dummy_io else {}
    xT_e = nc.dram_tensor("xT", [D, NT], BF16, **ik)
    wqk_e = nc.dram_tensor("wqk", [D, 4 * HD], BF16, **ik)
    bqk_e = nc.dram_tensor("bqk", [4 * HD], F32, **ik)
    DC2 = HPC * (HD + 1)         # v channels incl. the ones column (258)
    wv_e = nc.dram_tensor("wv", [D, DC2], BF16, **ik)
    bv_e = nc.dram_tensor("bv", [DC2], F32, **ik)
    cos_e = nc.dram_tensor("cosT", [HD, T], BF16, **ik)
    sin_e = nc.dram_tensor("sinT", [HD, T], BF16, **ik)
    masks_e = nc.dram_tensor("masks", [4, 128, 512], BF16, **ik)
    owT_e = nc.dram_tensor("owT", [D, D], BF16, **ik)
    ob_e = nc.dram_tensor("ob", [D], F32, **ik)
    out_e = nc.dram_tensor("out", [B * SB, D], F32, kind="ExternalOutput")

    with tile.TileContext(nc) as tc:
      for _it in range(n_loop):
        with tc.tile_pool(name=f"persist{_it}", bufs=1) as pp, \
             tc.tile_pool(name=f"xb{_it}", bufs=2) as xp, \
             tc.tile_pool(name=f"dram{_it}", bufs=1, space="DRAM") as dp:
            # ---- persistent tiles ----
            # weights: wqk in 4 column slices for fast start
            wqk_sb = pp.tile([128, KT, 4 * HD], BF16, tag="wqk", name="wqk")
            wqkr = wqk_e.rearrange("(kt p) f -> p kt f", p=128)
            bqk = pp.tile([128, 4], F32, tag="bqk", name="bqk")
            nc.scalar.dma_start(bqk[:], bqk_e.rearrange("(m p) -> p m", p=128))
            for q in range(4):
                nc.scalar.dma_start(
                    wqk_sb[:, bass.ts(q, KT // 4), 0:HD],
                    wqkr[:, bass.ts(q, KT // 4), 0:HD])
            for m in range(1, 4):
                nc.scalar.dma_start(
                    wqk_sb[:, :, bass.ts(m, HD)], wqkr[:, :, bass.ts(m, HD)])
            wv_sb = pp.tile([128, KT, DC2], BF16, tag="wv", name="wv")
            nc.scalar.dma_start(wv_sb[:], wv_e.rearrange("(kt p) f -> p kt f", p=128))
            cos_sb = pp.tile([128, T], BF16, tag="cos", name="cos")
            nc.scalar.dma_start(cos_sb[:], cos_e[:])
            sin_sb = pp.tile([128, T], BF16, tag="sin", name="sin")
            nc.scalar.dma_start(sin_sb[:], sin_e[:])
            masks = pp.tile([128, 4, 512], BF16, tag="masks", name="masks")
            nc.gpsimd.dma_start(masks[:], masks_e.rearrange("m p c -> p m c"))
            obB = pp.tile([128, D], F32, tag="obB", name="obB")
            bvB = pp.tile([128, DC2], F32, tag="bvB", name="bvB")

            # AllToAll staging: b0 single 1MB collective; b1 split per head-half.
            # Dedicated DRAM tensors (not pool tiles): pool-arena reuse would
            # alias them and serialize stores behind unrelated collectives.
            Zb0 = nc.dram_tensor(f"Zb0_{_it}", [N_CORES, HPC, HD, SB], BF16)[:]
            ZGb0 = nc.dram_tensor(f"ZGb0_{_it}", [N_CORES, HPC, HD, SB], BF16)[:]
            # batch-1 reshard split per head-half: the h0 AllToAll fires as
            # soon as attn(b1,h0) lands, overlapping attn(b1,h1) + pass 1.
            Zb1 = [nc.dram_tensor(f"Zb1_{i}_{_it}", [N_CORES, HD, SB], BF16)[:]
                   for i in range(HPC)]
            ZGb1 = [nc.dram_tensor(f"ZGb1_{i}_{_it}", [N_CORES, HD, SB], BF16)[:]
                    for i in range(HPC)]
            accbig = pp.tile([128, 2 * EB, 512], BF16, tag="accbig", name="accbig")
            zg_b0 = pp.tile([128, HPC, N_CORES, SB], BF16, tag="zg_b0", name="zg_b0")
            zg_b1 = pp.tile([128, HPC, N_CORES, SB], BF16, tag="zg_b1", name="zg_b1")

            def qkv_block(b, blk, qk, v_sb, tp, ps, xb=None):
                """Project tokens [b*T + blk*512 ...+512) -> qk slices + v."""
                tsl = bass.ts(blk, 512)                  # within-batch token slice
                gsl = bass.ds(b * T + blk * 512, 512)    # global token slice
                if xb is None:
                    xb = xp.tile([128, KT, 512], BF16, tag="xb", name="xb")
                    xTr = xT_e.rearrange("(kt p) t -> p kt t", p=128)
                    if b == 0 and blk == 0:
                        # finest-grained first load so the PE starts early
                        for q in range(4):
                            nc.sync.dma_start(xb[:, bass.ts(q, KT // 4), :],
                                              xTr[:, bass.ts(q, KT // 4), gsl])
                    else:
                        nc.sync.dma_start(xb[:, :KT // 2, :], xTr[:, :KT // 2, gsl])
                        nc.sync.dma_start(xb[:, KT // 2:, :], xTr[:, KT // 2:, gsl])
                for m in range(4):
                    psqk = ps.tile([128, 512], F32, tag="ps", name="ps")
                    for kt in range(KT):
                        nc.tensor.matmul(
                            psqk[:],
                            wqk_sb[:, kt, bass.ts(m, 128)],
                            xb[:, kt, :],
                            start=(kt == 0),
                            stop=(kt == KT - 1),
                        )
                    nc.scalar.activation(
                        qk[m][:, tsl], psqk[:], AF.Identity,
                        bias=bqk[:, m:m + 1], scale=1.0,
                    )
                    # RoPE in place: qk = qk*cos + swap(qk)*s2  (s2 = sin with
                    # first half negated, host-prepared; swap via SBUF DMA).
                    qm = qk[m][:, tsl]
                    qsw = tp.tile([128, 512], BF16, tag="qsw", name="qsw")
                    nc.scalar.dma_start(qsw[0:64, :], qm[64:128, :])
                    nc.scalar.dma_start(qsw[64:128, :], qm[0:64, :])
                    nc.vector.tensor_mul(qsw[:], qsw[:], sin_sb[:, tsl])
                    nc.vector.tensor_mul(qm, qm, cos_sb[:, tsl])
                    nc.vector.tensor_add(qm, qm, qsw[:])
                for tt in range(4):
                    psv = ps.tile([128, 512], F32, tag="ps", name="ps")
                    for kt in range(KT):
                        nc.tensor.matmul(
                            psv[:, :DC2],
                            xb[:, kt, bass.ts(tt, 128)],
                            wv_sb[:, kt, :],
                            start=(kt == 0),
                            stop=(kt == KT - 1),
                        )
                    nc.vector.tensor_add(
                        v_sb[:, blk * 4 + tt, :, :],
                        psv[:, :DC2].rearrange("p (h f) -> p h f", h=HPC),
                        bvB[:].rearrange("p (h f) -> p h f", h=HPC))

            def attn_rowblock(b, hh, rb, qk, v_sb, tp2, tpo, tpr, ps2, psA,
                              z_store):
                """One 512-query row block of causal attention for (b, hh).
                PV computes [q, vf+1]: the appended ones-column of v gives the
                softmax denominator; normalize via per-partition scalar, then
                DMA-transpose back to [vf, q] for the AllToAll layout."""
                rsl = bass.ts(rb, 512)                   # within-batch queries
                qT = qk[hh]
                kTt = qk[2 + hh]
                # one full PSUM bank per accumulation group: two interleaved
                # matmul accumulation groups sharing a bank corrupt each other
                # on hardware
                pvs = [psA.tile([128, 512], F32, tag=f"pv{qs}", name=f"pv{qs}")
                       for qs in range(4)]
                njt = 4 * rb + 4
                for jc in range(njt // 2):
                    pss = ps2.tile([128, 1024], F32, tag="pss", name="pss")
                    for half in range(2):
                        jt = 2 * jc + half
                        nc.tensor.matmul(
                            pss[:, bass.ts(half, 512)],
                            kTt[:, bass.ds(jt * 128, 128)],
                            qT[:, rsl],
                            start=True, stop=True,
                        )
                    pT = tp2.tile([128, 1024], BF16, tag="pT", name="pT")
                    nc.scalar.activation(pT[:], pss[:], AF.Exp, scale=SCALE)
                    m = 2 * jc - 4 * rb
                    if m >= 0:
                        nc.vector.tensor_mul(
                            pT[:], pT[:],
                            masks[:, m:m + 2, :].rearrange("p a b -> p (a b)"),
                        )
                    for half in range(2):
                        jt = 2 * jc + half
                        for qs in range(4):
                            nc.tensor.matmul(
                                pvs[qs][:, 0:HD + 1],
                                pT[:, bass.ds(half * 512 + qs * 128, 128)],
                                v_sb[:, jt, hh, :],
                                start=(jt == 0),
                                stop=(jt == njt - 1),
                            )
                oTt = tpo.tile([128, 4, 128], BF16, tag="oTt", name="oTt")
                for qs in range(4):
                    rq = tpr.tile([128, 1], F32, tag="rq", name="rq")
                    nc.vector.reciprocal(rq[:], pvs[qs][:, HD:HD + 1])
                    oN = tpr.tile([128, 128], BF16, tag="oN", name="oN")
                    nc.vector.tensor_scalar_mul(oN[:], pvs[qs][:, 0:HD], rq[:])
                    nc.sync.dma_start_transpose(oTt[:, qs, :], oN[:])
                z_store(rb, oTt)

            def attn_batch_head(b, hh, qk, v_sb, tp2, tpo, tpr, ps2, psA,
                                z_store, between=None):
                for rb in range(RB):
                    attn_rowblock(b, hh, rb, qk, v_sb, tp2, tpo, tpr, ps2,
                                  psA, z_store)
                    if between is not None and rb in between:
                        between[rb]()

            # ================= batch 0 =================
            with tc.tile_pool(name=f"b0{_it}", bufs=1) as bp0, \
                 tc.tile_pool(name=f"b0t{_it}", bufs=3) as tp0:
                qk0 = [bp0.tile([128, T], BF16, tag=f"qk{m}", name=f"b0qk{m}")
                       for m in range(4)]
                v0 = bp0.tile([128, T // 128, HPC, HD + 1], BF16, tag="v",
                              name="b0v")
                # bias loads + partition broadcasts (gpsimd queue, startup)
                bv1 = bp0.tile([1, DC2], F32, tag="bv1", name="bv1")
                nc.scalar.dma_start(bv1[:], bv_e[None, :])
                nc.gpsimd.partition_broadcast(bvB[:], bv1[:])
                ob1 = bp0.tile([1, D], F32, tag="ob1", name="ob1")
                nc.gpsimd.dma_start(ob1[:], ob_e[None, :])
                nc.gpsimd.partition_broadcast(obB[:], ob1[:])
                with tc.tile_pool(name=f"b0ps{_it}", bufs=6, space="PSUM") as ps0:
                    for blk in range(BLK):
                        qkv_block(0, blk, qk0, v0, tp0, ps0)

                with tc.tile_pool(name=f"a0t{_it}", bufs=3) as tp2, \
                     tc.tile_pool(name=f"a0o2{_it}", bufs=6) as tpo, \
                     tc.tile_pool(name=f"a0r2{_it}", bufs=8) as tpr, \
                     tc.tile_pool(name=f"a0s{_it}", bufs=2, space="PSUM") as ps2, \
                     tc.tile_pool(name=f"a0A{_it}", bufs=1, space="PSUM") as psA:
                    def z0_store(hh):
                        def store(rb, oTt):
                            nc.sync.dma_start(
                                Zb0[2 * rb:2 * rb + 2, hh, :, :].rearrange(
                                    "j d t -> d j t"),
                                oTt[:, :, :].rearrange("p (j a) b -> p j (a b)",
                                                       j=2),
                            )
                        return store
                    # prefetch b1's first x block while attention runs
                    def pf_xb4():
                        xb4 = xp.tile([128, KT, 512], BF16, tag="xb", name="xb")
                        xTr = xT_e.rearrange("(kt p) t -> p kt t", p=128)
                        gsl = bass.ds(T, 512)
                        nc.sync.dma_start(xb4[:, :KT // 2, :], xTr[:, :KT // 2, gsl])
                        nc.sync.dma_start(xb4[:, KT // 2:, :], xTr[:, KT // 2:, gsl])
                        pf_xb4.tile = xb4
                    attn_batch_head(0, 0, qk0, v0, tp2, tpo, tpr, ps2, psA,
                                    z0_store(0), between={1: pf_xb4})
                    attn_batch_head(0, 1, qk0, v0, tp2, tpo, tpr, ps2, psA,
                                    z0_store(1))

            if fake_cc:
                nc.sync.dma_start(ZGb0[:], Zb0[:])
            else:
                nc.gpsimd.collective_compute(
                    "AllToAll", ALU.bypass,
                    replica_groups=[list(range(N_CORES))],
                    ins=[Zb0[:]], outs=[ZGb0[:]],
                )
            # out_w preload (8MB) on the gpsimd queue; lands in SBUF freed by
            # the batch-0 pools, well before the out-projection needs it.
            with tc.tile_pool(name=f"ow{_it}", bufs=1) as owp:
                # ow contraction-tile order kt = h = 2*s + i (head index);
                # loaded in quarters on the sync queue, interleaved with the
                # b1 x-block prefetches (hardware DGE; the swdge path on
                # gpsimd is software-paced and slow on real hardware)
                ow = owp.tile([128, KT, D], BF16, tag="ow", name="ow")
                owr = owT_e.rearrange("(kt p) f -> p kt f", p=128)

                def ow_load(q):
                    nc.sync.dma_start(
                        ow[:, bass.ts(q, KT // 4), :], owr[:, bass.ts(q, KT // 4), :])

                # ================= batch 1 =================
                with tc.tile_pool(name=f"b1{_it}", bufs=1) as bp1, \
                     tc.tile_pool(name=f"b1t{_it}", bufs=3) as tp1:
                    qk1 = [bp1.tile([128, T], BF16, tag=f"qk{m}", name=f"b1qk{m}")
                           for m in range(4)]
                    v1 = bp1.tile([128, T // 128, HPC, HD + 1], BF16, tag="v",
                                  name="b1v")
                    with tc.tile_pool(name=f"b1ps{_it}", bufs=6, space="PSUM") as ps1:
                      for blk in range(BLK):
                        if blk == 0:
                            # first block's x was prefetched during attn(b0)
                            qkv_block(1, 0, qk1, v1, tp1, ps1, xb=pf_xb4.tile)
                            ow_load(0)
                        else:
                            qkv_block(1, blk, qk1, v1, tp1, ps1)
                            ow_load(blk)

                    # zg_b0 pull on the (now idle) sync queue; waits on CC(b0)
                    for i in range(HPC):
                        nc.sync.dma_start(
                            zg_b0[:, i, :, :],
                            ZGb0[:, i, :, :].rearrange("s d t -> d s t"))

                    with tc.tile_pool(name=f"a1t{_it}", bufs=3) as tp2, \
                         tc.tile_pool(name=f"a1o2{_it}", bufs=6) as tpo, \
                         tc.tile_pool(name=f"a1r2{_it}", bufs=8) as tpr, \
                         tc.tile_pool(name=f"a1s{_it}", bufs=2, space="PSUM") as ps2, \
                         tc.tile_pool(name=f"a1A{_it}", bufs=1, space="PSUM") as psA:
                        def z1_store(hh):
                            def store(rb, oTt):
                                nc.sync.dma_start(
                                    Zb1[hh][2 * rb:2 * rb + 2, :, :].rearrange(
                                        "j d t -> d j t"),
                                    oTt[:, :, :].rearrange("p (j a) b -> p j (a b)",
                                                           j=2),
                                )
                            return store
                        for hh in range(HPC):
                            attn_batch_head(1, hh, qk1, v1, tp2, tpo, tpr,
                                            ps2, psA, z1_store(hh))

                for hh in range(HPC):
                    if fake_cc:
                        nc.sync.dma_start(ZGb1[hh][:], Zb1[hh][:])
                    else:
                        nc.gpsimd.collective_compute(
                            "AllToAll", ALU.bypass,
                            replica_groups=[list(range(N_CORES))],
                            ins=[Zb1[hh][:]], outs=[ZGb1[hh][:]],
                        )

                # ================= out projection =================
                with tc.tile_pool(name=f"p4t{_it}", bufs=4) as tp4, \
                     tc.tile_pool(name=f"p4ps{_it}", bufs=4, space="PSUM") as ps4:
                    # pass 1: batch-0 tokens, full 2048-feature contraction
                    for e in range(EB):
                        for tt in range(SB // 128):
                            pso4 = ps4.tile([128, 512], F32, tag="ps4", name="ps4")
                            for zt in range(KT):
                                nc.tensor.matmul(
                                    pso4[:],
                                    zg_b0[:, zt % 2, zt // 2, bass.ts(tt, 128)],
                                    ow[:, zt, bass.ts(e, 512)],
                                    start=(zt == 0),
                                    stop=(zt == KT - 1),
                                )
                            of = tp4.tile([128, 512], F32, tag="of", name="of")
                            nc.vector.tensor_add(of[:], pso4[:], obB[:, bass.ts(e, 512)])
                            nc.sync.dma_start(
                                out_e[bass.ds(tt * 128, 128), bass.ts(e, 512)], of[:])
                    # zg_b1 pulls on the scalar queue (idle after attention)
                    nc.scalar.dma_start(
                        zg_b1[:, 0, :, :],
                        ZGb1[0][:].rearrange("s d t -> d s t"))
                    # pass 2a: batch-1 tokens, h0 (even-head) half of contraction
                    for e in range(EB):
                        for tt in range(SB // 128):
                            pso4 = ps4.tile([128, 512], F32, tag="ps4", name="ps4")
                            for s in range(N_CORES):
                                nc.tensor.matmul(
                                    pso4[:],
                                    zg_b1[:, 0, s, bass.ts(tt, 128)],
                                    ow[:, 2 * s, bass.ts(e, 512)],
                                    start=(s == 0),
                                    stop=(s == N_CORES - 1),
                                )
                            nc.vector.tensor_copy(
                                accbig[:, e * (SB // 128) + tt, :], pso4[:])
                    nc.scalar.dma_start(
                        zg_b1[:, 1, :, :],
                        ZGb1[1][:].rearrange("s d t -> d s t"))
                    # pass 2b: h1 (odd-head) half + accumulated sum + bias
                    for e in range(EB):
                        for tt in range(SB // 128):
                            pso4 = ps4.tile([128, 512], F32, tag="ps4", name="ps4")
                            for s in range(N_CORES):
                                nc.tensor.matmul(
                                    pso4[:],
                                    zg_b1[:, 1, s, bass.ts(tt, 128)],
                                    ow[:, 2 * s + 1, bass.ts(e, 512)],
                                    start=(s == 0),
                                    stop=(s == N_CORES - 1),
                                )
                            mid = tp4.tile([128, 512], F32, tag="mid", name="mid")
                            nc.vector.tensor_add(
                                mid[:], pso4[:], accbig[:, e * (SB // 128) + tt, :])
                            of = tp4.tile([128, 512], F32, tag="of", name="of")
                            nc.vector.tensor_add(of[:], mid[:], obB[:, bass.ts(e, 512)])
                            nc.sync.dma_start(
                                out_e[bass.ds(SB + tt * 128, 128), bass.ts(e, 512)],
                                of[:])

    nc.compile()          # Bacc pass pipeline (library loads, nop fusion, regs)
    legalize_waits(nc)    # must run after all nop-fusion passes
    bass.Bass.finalize(nc)  # freeze without re-running Bacc compile
    return nc


def _prep_inputs(x, rope_cos, rope_sin, qkv_w, qkv_b, out_w, out_b, B, T, D, H):
    HD = D // H
    NT = B * T
    HPC = H // N_CORES
    bf = ml_dtypes.bfloat16

    x2 = np.ascontiguousarray(x.reshape(NT, D).T).astype(bf)           # [D, NT]
    cosT = np.ascontiguousarray(rope_cos[0, 0].T).astype(bf)           # [HD, T]
    s2 = rope_sin[0, 0].T.copy()
    s2[:HD // 2] *= -1.0
    sinT = np.ascontiguousarray(s2).astype(bf)
    owT = np.ascontiguousarray(out_w.T).astype(bf)                      # [D, D]
    ob = out_b.astype(np.float32)

    c_grid = np.arange(512)[None, :]
    p_grid = np.arange(128)[:, None]
    masks = np.stack(
        [(c_grid >= 128 * m + p_grid) for m in range(4)]
    ).astype(bf)                                                        # [4,128,512]

    in_maps = []
    for c in range(N_CORES):
        heads = [HPC * c + i for i in range(HPC)]
        q_rows = np.concatenate([qkv_w[h * HD:(h + 1) * HD] for h in heads])
        k_rows = np.concatenate([qkv_w[D + h * HD:D + (h + 1) * HD] for h in heads])
        v_rows = np.concatenate(
            [np.concatenate(
                [qkv_w[2 * D + h * HD:2 * D + (h + 1) * HD],
                 np.zeros((1, D), qkv_w.dtype)])
             for h in heads])
        wqk = np.ascontiguousarray(np.concatenate([q_rows, k_rows]).T).astype(bf)
        wv = np.ascontiguousarray(v_rows.T).astype(bf)
        bq = np.concatenate([qkv_b[h * HD:(h + 1) * HD] for h in heads])
        bk = np.concatenate([qkv_b[D + h * HD:D + (h + 1) * HD] for h in heads])
        bqk = np.concatenate([bq, bk]).astype(np.float32)
        bv = np.concatenate(
            [np.concatenate(
                [qkv_b[2 * D + h * HD:2 * D + (h + 1) * HD], np.ones(1)])
             for h in heads]
        ).astype(np.float32)
        in_maps.append({
            "xT": x2, "wqk": wqk, "bqk": bqk, "wv": wv, "bv": bv,
            "cosT": cosT, "sinT": sinT, "masks": masks,
            "owT": owT, "ob": ob,
        })
    return in_maps


_NC_CACHE = {}


def kernel(x, rope_cos, rope_sin, qkv_w, qkv_b, out_w, out_b):
    B, T, D = x.shape
    H = 16
    NT = B * T
    SB = T // N_CORES
    key = (B, T, D, H)
    if key not in _NC_CACHE:
        _NC_CACHE[key] = build_nc(B, T, D, H)
    nc = _NC_CACHE[key]
    in_maps = _prep_inputs(
        np.asarray(x), np.asarray(rope_cos), np.asarray(rope_sin),
        np.asarray(qkv_w), np.asarray(qkv_b), np.asarray(out_w),
        np.asarray(out_b), B, T, D, H,
    )
    res = run_bass_kernel_spmd(nc, in_maps, core_ids=list(range(N_CORES)))
    out = np.empty((NT, D), np.float32)
    for c in range(N_CORES):
        r = res.results[c]["out"]                       # [2*SB, D]
        out[c * SB:(c + 1) * SB] = r[:SB]               # batch-0 token slice
        out[T + c * SB:T + (c + 1) * SB] = r[SB:]       # batch-1 token slice
    return out.reshape(B, T, D)



# revision 41
# speedup vs baseline: 1.0109x; 1.0109x over previous
"""Causal self-attention with RoPE on 8 Trainium2 NeuronCores.

Sharding: tensor-parallel over heads (2 heads/core) through QKV projection,
RoPE and attention; AllToAll reshards attention output from head-split to
token-split; out-projection is token-parallel with full out_w per core.

Schedule: per-batch interleave so the resharding collectives overlap compute:
  QKV(b0) -> attn(b0,h0),attn(b0,h1) -> CC(b0) ->
  QKV(b1) -> attn(b1,h0) -> CC(b1,h0) -> attn(b1,h1) -> CC(b1,h1) ->
  out-proj pass1 (b0 tokens, full contraction)
  out-proj pass2a (b1 tokens, h0-half contraction) + pass2b (h1-half).
Each core ends up owning tokens [c*256,(c+1)*256) of each batch element.

Layouts (per core, f = feature, t = token, d = contraction):
  xT   [D, NT]    input transposed (d on partitions) - rhs/lhsT for projections
  q/k  [128, T]   per-(head,batch), head-dim on partitions: proj out [f, t]
  v    [T, 256]   token-major: proj out [t, f]
  sT   [j, r]     scores transposed: lhsT=kT-tile, rhs=qT-block
  pT   [j, r]     exp(scores*scale) bf16
  oT   [dv, r]    PV: lhsT=v-tile [j, dv], rhs=pT [j, r]
  denom[1, r]     ones-matmul over fp32-accumulated pT (PE PSUM accumulation)
  recipB          K=1 PE matmul broadcasts 1/denom to 128 partitions (PSUM)
  out  [t, e]     out-proj: lhsT=zg-tile [dv, t], rhs=ow [dv, e]
"""
import math
import numpy as np
import ml_dtypes

import concourse.bass as bass
import concourse.mybir as mybir
import concourse.tile as tile
from concourse import bacc
from concourse.bass_utils import run_bass_kernel_spmd

F32 = mybir.dt.float32
BF16 = mybir.dt.bfloat16
AF = mybir.ActivationFunctionType
ALU = mybir.AluOpType

N_CORES = 8


def legalize_waits(nc, max_waits=1):
    """This walrus build only encodes one sync-wait per TPB instruction.
    Move extra waits emitted by Tile onto same-engine NoOps inserted
    immediately before the instruction."""
    n_split = 0
    for fn in nc.m.functions:
        for bb in fn.blocks:
            new_insts = []
            for inst in bb.instructions:
                si = getattr(inst, "sync_info", None)
                waits = list(si.on_wait) if si is not None and si.on_wait else []
                if len(waits) > max_waits and type(inst).__name__ != "InstNoOp":
                    extra, keep = waits[:-max_waits], waits[-max_waits:]
                    for k, w in enumerate(extra):
                        nop = mybir.InstNoOp(
                            name=f"{inst.name}_waitnop{k}",
                            engine=inst.engine,
                            ins=[],
                            outs=[],
                            sync_info=mybir.SyncInfo(on_wait=[w], on_update=[]),
                        )
                        nc.register_instruction(nop)
                        new_insts.append(nop)
                    inst.sync_info = mybir.SyncInfo(
                        on_wait=keep, on_update=list(si.on_update)
                    )
                    n_split += 1
                new_insts.append(inst)
            bb.instructions = new_insts
    return n_split


def build_nc(B=2, T=2048, D=2048, H=16, fake_cc=False, n_loop=1, dummy_io=False,
             cc_nodep=False, cc_split4=False, cc_merge1=False, stop_after=None,
             ilv=False):
    if ilv:
        cc_merge1 = True         # single merged b1 collective
    # stop_after: None | "qkv0" | "attn0" | "qkv1" | "attn1"  (HW phase timing)
    SA = {None: 99, "qkv0": 0, "attn0": 1, "qkv1": 2, "attn1": 3}[stop_after]
    HD = D // H                  # 128, head dim
    NT = B * T                   # total tokens
    HPC = H // N_CORES           # heads per core (2)
    DC = HPC * HD                # head channels per core (256)
    KT = D // 128                # contraction tiles for projections (16)
    RB = T // 512                # 512-token row blocks per batch element (4)
    BLK = T // 512               # 512-token qkv blocks per batch element (4)
    SB = T // N_CORES            # tokens per core per batch after AllToAll (256)
    EB = D // 512                # 512-wide out-feature blocks (4)
    SCALE = 1.0 / math.sqrt(HD)

    nc = bacc.Bacc("TRN2", target_bir_lowering=False, debug=False, num_devices=N_CORES)
    ik = {"kind": "ExternalInput"} if not dummy_io else {}
    xT_e = nc.dram_tensor("xT", [D, NT], BF16, **ik)
    wqk_e = nc.dram_tensor("wqk", [D, 4 * HD], BF16, **ik)
    bqk_e = nc.dram_tensor("bqk", [4 * HD], F32, **ik)
    DC2 = HPC * (HD + 1)         # v channels incl. the ones column (258)
    wv_e = nc.dram_tensor("wv", [D, DC2], BF16, **ik)
    bv_e = nc.dram_tensor("bv", [DC2], F32, **ik)
    cos_e = nc.dram_tensor("cosT", [HD, T], BF16, **ik)
    sin_e = nc.dram_tensor("sinT", [HD, T], BF16, **ik)
    masks_e = nc.dram_tensor("masks", [4, 128, 512], BF16, **ik)
    owT_e = nc.dram_tensor("owT", [D, D], BF16, **ik)
    ob_e = nc.dram_tensor("ob", [D], F32, **ik)
    out_e = nc.dram_tensor("out", [B * SB, D], F32, kind="ExternalOutput")

    with tile.TileContext(nc) as tc:
      for _it in range(n_loop):
        with tc.tile_pool(name=f"persist{_it}", bufs=1) as pp, \
             tc.tile_pool(name=f"xb{_it}", bufs=2) as xp, \
             tc.tile_pool(name=f"dram{_it}", bufs=1, space="DRAM") as dp:
            # ---- persistent tiles ----
            # weights: wqk in 4 column slices for fast start
            bqk = pp.tile([128, 4], F32, tag="bqk", name="bqk")
            nc.scalar.dma_start(bqk[:], bqk_e.rearrange("(m p) -> p m", p=128))
            wqkr = wqk_e.rearrange("(kt p) f -> p kt f", p=128)

            def load_weights(wp):
                wqk_sb = wp.tile([128, KT, 4 * HD], BF16, tag="wqk", name="wqk")
                for q in range(4):
                    nc.scalar.dma_start(
                        wqk_sb[:, bass.ts(q, KT // 4), 0:HD],
                        wqkr[:, bass.ts(q, KT // 4), 0:HD])
                for m in range(1, 4):
                    nc.scalar.dma_start(
                        wqk_sb[:, :, bass.ts(m, HD)], wqkr[:, :, bass.ts(m, HD)])
                wv_sb = wp.tile([128, KT, DC2], BF16, tag="wv", name="wv")
                nc.scalar.dma_start(
                    wv_sb[:], wv_e.rearrange("(kt p) f -> p kt f", p=128))
                cos_sb = wp.tile([128, T], BF16, tag="cos", name="cos")
                nc.scalar.dma_start(cos_sb[:], cos_e[:])
                sin_sb = wp.tile([128, T], BF16, tag="sin", name="sin")
                nc.scalar.dma_start(sin_sb[:], sin_e[:])
                return wqk_sb, wv_sb, cos_sb, sin_sb

            if not ilv:
                wqk_sb, wv_sb, cos_sb, sin_sb = load_weights(pp)
            masks = pp.tile([128, 4, 512], BF16, tag="masks", name="masks")
            nc.gpsimd.dma_start(masks[:], masks_e.rearrange("m p c -> p m c"))
            obB = pp.tile([128, D], F32, tag="obB", name="obB")
            bvB = pp.tile([128, DC2], F32, tag="bvB", name="bvB")

            # AllToAll staging: b0 single 1MB collective; b1 split per head-half.
            # Dedicated DRAM tensors (not pool tiles): pool-arena reuse would
            # alias them and serialize stores behind unrelated collectives.
            if cc_split4:
                Zb0h = [nc.dram_tensor(f"Zb0h_{i}_{_it}", [N_CORES, HD, SB], BF16)[:]
                        for i in range(HPC)]
                ZGb0h = [nc.dram_tensor(f"ZGb0h_{i}_{_it}", [N_CORES, HD, SB], BF16)[:]
                         for i in range(HPC)]
            else:
                Zb0 = nc.dram_tensor(f"Zb0_{_it}", [N_CORES, HPC, HD, SB], BF16)[:]
                ZGb0 = nc.dram_tensor(f"ZGb0_{_it}", [N_CORES, HPC, HD, SB], BF16)[:]
            # batch-1 reshard split per head-half: the h0 AllToAll fires as
            # soon as attn(b1,h0) lands, overlapping attn(b1,h1) + pass 1.
            if cc_merge1:
                Zb1m = nc.dram_tensor(f"Zb1m_{_it}", [N_CORES, HPC, HD, SB], BF16)
                ZGb1m = nc.dram_tensor(f"ZGb1m_{_it}", [N_CORES, HPC, HD, SB], BF16)
            Zb1 = [nc.dram_tensor(f"Zb1_{i}_{_it}", [N_CORES, HD, SB], BF16)[:]
                   for i in range(HPC)]
            ZGb1 = [nc.dram_tensor(f"ZGb1_{i}_{_it}", [N_CORES, HD, SB], BF16)[:]
                    for i in range(HPC)]
            accbig = pp.tile([128, 2 * EB, 512], BF16, tag="accbig", name="accbig")
            zg_b0 = pp.tile([128, HPC, N_CORES, SB], BF16, tag="zg_b0", name="zg_b0")
            zg_b1 = pp.tile([128, HPC, N_CORES, SB], BF16, tag="zg_b1", name="zg_b1")

            def qkv_block(b, blk, qk, v_sb, tp, ps, xb=None, xpool=None):
                """Project tokens [b*T + blk*512 ...+512) -> qk slices + v."""
                tsl = bass.ts(blk, 512)                  # within-batch token slice
                gsl = bass.ds(b * T + blk * 512, 512)    # global token slice
                if xb is None:
                    xb = (xpool or xp).tile([128, KT, 512], BF16, tag="xb",
                                            name="xb")
                    xTr = xT_e.rearrange("(kt p) t -> p kt t", p=128)
                    if b == 0 and blk == 0:
                        # finest-grained first load so the PE starts early
                        for q in range(4):
                            nc.sync.dma_start(xb[:, bass.ts(q, KT // 4), :],
                                              xTr[:, bass.ts(q, KT // 4), gsl])
                    else:
                        nc.sync.dma_start(xb[:, :KT // 2, :], xTr[:, :KT // 2, gsl])
                        nc.sync.dma_start(xb[:, KT // 2:, :], xTr[:, KT // 2:, gsl])
                for m in range(4):
                    psqk = ps.tile([128, 512], F32, tag="ps", name="ps")
                    for kt in range(KT):
                        nc.tensor.matmul(
                            psqk[:],
                            wqk_sb[:, kt, bass.ts(m, 128)],
                            xb[:, kt, :],
                            start=(kt == 0),
                            stop=(kt == KT - 1),
                        )
                    # bias-add on DVE (keeps ACT free for attention exps and
                    # avoids Identity<->Exp activation-table switches)
                    nc.vector.tensor_scalar_add(
                        qk[m][:, tsl], psqk[:], bqk[:, m:m + 1])
                    # RoPE in place: qk = qk*cos + swap(qk)*s2  (s2 = sin with
                    # first half negated, host-prepared; swap via SBUF DMA).
                    qm = qk[m][:, tsl]
                    qsw = tp.tile([128, 512], BF16, tag="qsw", name="qsw")
                    # swap halves on the sync queue: scalar-queue DMAs stall
                    # the exp stream that shares that queue
                    nc.sync.dma_start(qsw[0:64, :], qm[64:128, :])
                    nc.sync.dma_start(qsw[64:128, :], qm[0:64, :])
                    nc.vector.tensor_mul(qsw[:], qsw[:], sin_sb[:, tsl])
                    nc.vector.tensor_mul(qm, qm, cos_sb[:, tsl])
                    nc.vector.tensor_add(qm, qm, qsw[:])
                for tt in range(4):
                    psv = ps.tile([128, 512], F32, tag="ps", name="ps")
                    for kt in range(KT):
                        nc.tensor.matmul(
                            psv[:, :DC2],
                            xb[:, kt, bass.ts(tt, 128)],
                            wv_sb[:, kt, :],
                            start=(kt == 0),
                            stop=(kt == KT - 1),
                        )
                    nc.vector.tensor_add(
                        v_sb[:, blk * 4 + tt, :, :],
                        psv[:, :DC2].rearrange("p (h f) -> p h f", h=HPC),
                        bvB[:].rearrange("p (h f) -> p h f", h=HPC))

            def attn_rowblock(b, hh, rb, qk, v_sb, tp2, tpo, tpr, ps2, psA,
                              z_store):
                """One 512-query row block of causal attention for (b, hh).
                PV computes [q, vf+1]: the appended ones-column of v gives the
                softmax denominator; normalize via per-partition scalar, then
                DMA-transpose back to [vf, q] for the AllToAll layout."""
                rsl = bass.ts(rb, 512)                   # within-batch queries
                qT = qk[hh]
                kTt = qk[2 + hh]
                # one full PSUM bank per accumulation group: two interleaved
                # matmul accumulation groups sharing a bank corrupt each other
                # on hardware
                pvs = [psA.tile([128, 512], F32, tag=f"pv{qs}", name=f"pv{qs}")
                       for qs in range(4)]
                njt = 4 * rb + 4

                def pv_emit(jt, pT):
                    # q-slices strictly above the diagonal are fully masked:
                    # their PV products are exactly zero - skip them. Each
                    # qs-group's accumulation then stops at its last
                    # participating key tile (jt == 4*rb + qs).
                    m = jt - 4 * rb
                    for qs in range(4):
                        if qs < m:
                            continue
                        nc.tensor.matmul(
                            pvs[qs][:, 0:HD + 1],
                            pT[:, bass.ds(qs * 128, 128)],
                            v_sb[:, jt, hh, :],
                            start=(jt == 0),
                            stop=(jt == 4 * rb + qs),
                        )
                prev = None
                for jt in range(njt):
                    m = jt - 4 * rb
                    moff = 128 * max(m, 0)
                    pss = ps2.tile([128, 512], F32, tag="pss", name="pss")
                    nc.tensor.matmul(
                        pss[:, moff:],
                        kTt[:, bass.ds(jt * 128, 128)],
                        qT[:, bass.ds(rb * 512 + moff, 512 - moff)],
                        start=True, stop=True,
                    )
                    pT = tp2.tile([128, 512], BF16, tag="pT", name="pT")
                    nc.scalar.activation(pT[:, moff:], pss[:, moff:],
                                         AF.Exp, scale=SCALE)
                    if m >= 0:
                        # only the on-diagonal 128-query slice needs masking
                        nc.vector.tensor_mul(
                            pT[:, moff:moff + 128], pT[:, moff:moff + 128],
                            masks[:, 0, 0:128])
                    # PV of the previous key tile: PE scores tile jt while the
                    # ACT exp of tile jt-1 is already done -> no PE stall
                    if prev is not None:
                        pv_emit(*prev)
                    prev = (jt, pT)
                pv_emit(*prev)
                oTt = tpo.tile([128, 4, 128], BF16, tag="oTt", name="oTt")
                for qs in range(4):
                    rq = tpr.tile([128, 1], F32, tag="rq", name="rq")
                    nc.vector.reciprocal(rq[:], pvs[qs][:, HD:HD + 1])
                    oN = tpr.tile([128, 128], BF16, tag="oN", name="oN")
                    nc.vector.tensor_scalar_mul(oN[:], pvs[qs][:, 0:HD], rq[:])
                    nc.sync.dma_start_transpose(oTt[:, qs, :], oN[:])
                z_store(rb, oTt)

            def attn_batch_head(b, hh, qk, v_sb, tp2, tpo, tpr, ps2, psA,
                                z_store, between=None):
                for rb in range(RB):
                    attn_rowblock(b, hh, rb, qk, v_sb, tp2, tpo, tpr, ps2,
                                  psA, z_store)
                    if between is not None and rb in between:
                        between[rb]()

            if ilv:
                # ======== fully-pipelined emission ========
                # One PE stream: qkv(b,blk) blocks interleaved with attention
                # rowblocks (which depend only on earlier qkv blocks), then the
                # out-projection split into source-halves. Exactly two
                # AllToAlls: CC(b0) mid-pipeline, CC(b1) at attention end with
                # the b0-projection passes left to hide it. ACT exp load is
                # spread over the whole qkv window so it never gates the PE.
                def cc_one(zin, zout):
                    if fake_cc:
                        nc.sync.dma_start(zout, zin)
                    else:
                        nc.gpsimd.collective_compute(
                            "AllToAll", ALU.bypass,
                            replica_groups=[list(range(N_CORES))],
                            ins=[zin], outs=[zout],
                        )

                with tc.tile_pool(name=f"ibq{_it}", bufs=1) as bq, \
                     tc.tile_pool(name=f"iqt{_it}", bufs=3) as tpq, \
                     tc.tile_pool(name=f"iat{_it}", bufs=3) as tp2, \
                     tc.tile_pool(name=f"iao{_it}", bufs=6) as tpo, \
                     tc.tile_pool(name=f"iar{_it}", bufs=8) as tpr, \
                     tc.tile_pool(name=f"ias{_it}", bufs=2, space="PSUM") as ps2, \
                     tc.tile_pool(name=f"iaA{_it}", bufs=1, space="PSUM") as psA:
                    qk0 = [bq.tile([128, T], BF16, tag=f"qk0{m}", name=f"b0qk{m}")
                           for m in range(4)]
                    v0 = bq.tile([128, T // 128, HPC, HD + 1], BF16, tag="v0",
                                 name="b0v")
                    qk1 = [bq.tile([128, T], BF16, tag=f"qk1{m}", name=f"b1qk{m}")
                           for m in range(4)]
                    v1 = bq.tile([128, T // 128, HPC, HD + 1], BF16, tag="v1",
                                 name="b1v")
                    qks, vss = [qk0, qk1], [v0, v1]
                    bv1 = bq.tile([1, DC2], F32, tag="bv1", name="bv1")
                    nc.scalar.dma_start(bv1[:], bv_e[None, :])
                    nc.gpsimd.partition_broadcast(bvB[:], bv1[:])
                    ob1 = bq.tile([1, D], F32, tag="ob1", name="ob1")
                    nc.gpsimd.dma_start(ob1[:], ob_e[None, :])
                    nc.gpsimd.partition_broadcast(obB[:], ob1[:])

                    def zst(b, hh):
                        def store(rb, oTt):
                            dst = (Zb0[2 * rb:2 * rb + 2, hh, :, :] if b == 0
                                   else Zb1m[2 * rb:2 * rb + 2, hh, :, :])
                            nc.sync.dma_start(
                                dst.rearrange("j d t -> d j t"),
                                oTt[:, :, :].rearrange(
                                    "p (j a) b -> p j (a b)", j=2),
                            )
                        return store

                    def arb(b, hh, rb):
                        attn_rowblock(b, hh, rb, qks[b], vss[b], tp2, tpo, tpr,
                                      ps2, psA, zst(b, hh))

                    with tc.tile_pool(name=f"iwq{_it}", bufs=1) as wq, \
                         tc.tile_pool(name=f"iqp{_it}", bufs=2,
                                      space="PSUM") as psq, \
                         tc.tile_pool(name=f"ixb{_it}", bufs=2) as xpi:
                        wqk_sb, wv_sb, cos_sb, sin_sb = load_weights(wq)
                        qkv_block(0, 0, qk0, v0, tpq, psq, xpool=xpi)
                        for k in range(4):
                            if k < 3:
                                qkv_block(0, k + 1, qk0, v0, tpq, psq, xpool=xpi)
                            else:
                                qkv_block(1, 0, qk1, v1, tpq, psq, xpool=xpi)
                            arb(0, 0, k)
                            arb(0, 1, k)
                            if k == 3:
                                cc_one(Zb0[:], ZGb0[:])
                                # zg_b0 pulls queue behind CC(b0) on the gpsimd
                                # queue and run during the b1 pipeline (~80us
                                # of slack before pass1a needs them)
                                for i in range(HPC):
                                    nc.gpsimd.dma_start(
                                        zg_b0[:, i, :, :],
                                        ZGb0[:, i, :, :].rearrange(
                                            "s d t -> d s t"))
                        for k in range(3):
                            qkv_block(1, k + 1, qk1, v1, tpq, psq, xpool=xpi)
                            arb(1, 0, k)
                            arb(1, 1, k)
                    # xb + qkv-psum arenas freed; out-proj weights stream in
                    # s-major halves so pass1a only waits on the first 4 MB
                    with tc.tile_pool(name=f"iow{_it}", bufs=1) as owp, \
                         tc.tile_pool(name=f"ip4{_it}", bufs=4) as tp4, \
                         tc.tile_pool(name=f"ip4s{_it}", bufs=2,
                                      space="PSUM") as ps4:
                        ow = owp.tile([128, N_CORES, HPC, D], BF16, tag="ow",
                                      name="ow")
                        owr = owT_e.rearrange("(s i p) f -> p s i f",
                                              p=128, i=HPC)
                        nc.sync.dma_start(ow[:, 0:4], owr[:, 0:4])
                        nc.sync.dma_start(ow[:, 4:8], owr[:, 4:8])
                        arb(1, 0, 3)
                        arb(1, 1, 3)
                        cc_one(Zb1m[:], ZGb1m[:])
                        # source-half s0..3 (both heads) pulled first: pass2a
                        # only waits on half the reshard payload
                        for sh in range(2):
                            for i in range(HPC):
                                nc.scalar.dma_start(
                                    zg_b1[:, i, 4 * sh:4 * sh + 4, :],
                                    ZGb1m[4 * sh:4 * sh + 4, i, :, :].rearrange(
                                        "s d t -> d s t"))

                        def proj_half(zg, sh, dst_store):
                            for e in range(EB):
                                for tt in range(SB // 128):
                                    pso4 = ps4.tile([128, 512], F32, tag="ps4",
                                                    name="ps4")
                                    for u in range(8):
                                        s, i = 4 * sh + u // 2, u % 2
                                        nc.tensor.matmul(
                                            pso4[:],
                                            zg[:, i, s, bass.ts(tt, 128)],
                                            ow[:, s, i, bass.ts(e, 512)],
                                            start=(u == 0),
                                            stop=(u == 7),
                                        )
                                    dst_store(e, tt, pso4)

                        def to_acc(e, tt, pso4):
                            nc.vector.tensor_copy(
                                accbig[:, e * (SB // 128) + tt, :], pso4[:])

                        def fin_store(tok0):
                            def store(e, tt, pso4):
                                mid = tp4.tile([128, 512], F32, tag="mid",
                                               name="mid")
                                nc.vector.tensor_add(
                                    mid[:], pso4[:],
                                    accbig[:, e * (SB // 128) + tt, :])
                                of = tp4.tile([128, 512], F32, tag="of",
                                              name="of")
                                nc.vector.tensor_add(
                                    of[:], mid[:], obB[:, bass.ts(e, 512)])
                                nc.sync.dma_start(
                                    out_e[bass.ds(tok0 + tt * 128, 128),
                                          bass.ts(e, 512)], of[:])
                            return store

                        proj_half(zg_b0, 0, to_acc)
                        proj_half(zg_b0, 1, fin_store(0))
                        proj_half(zg_b1, 0, to_acc)
                        proj_half(zg_b1, 1, fin_store(SB))
                continue

            # ================= batch 0 =================
            with tc.tile_pool(name=f"b0{_it}", bufs=1) as bp0, \
                 tc.tile_pool(name=f"b0t{_it}", bufs=3) as tp0:
                qk0 = [bp0.tile([128, T], BF16, tag=f"qk{m}", name=f"b0qk{m}")
                       for m in range(4)]
                v0 = bp0.tile([128, T // 128, HPC, HD + 1], BF16, tag="v",
                              name="b0v")
                # bias loads + partition broadcasts (gpsimd queue, startup)
                bv1 = bp0.tile([1, DC2], F32, tag="bv1", name="bv1")
                nc.scalar.dma_start(bv1[:], bv_e[None, :])
                nc.gpsimd.partition_broadcast(bvB[:], bv1[:])
                ob1 = bp0.tile([1, D], F32, tag="ob1", name="ob1")
                nc.gpsimd.dma_start(ob1[:], ob_e[None, :])
                nc.gpsimd.partition_broadcast(obB[:], ob1[:])
                with tc.tile_pool(name=f"b0ps{_it}", bufs=6, space="PSUM") as ps0:
                    for blk in range(BLK):
                        qkv_block(0, blk, qk0, v0, tp0, ps0)

                with tc.tile_pool(name=f"a0t{_it}", bufs=3) as tp2, \
                     tc.tile_pool(name=f"a0o2{_it}", bufs=6) as tpo, \
                     tc.tile_pool(name=f"a0r2{_it}", bufs=8) as tpr, \
                     tc.tile_pool(name=f"a0s{_it}", bufs=2, space="PSUM") as ps2, \
                     tc.tile_pool(name=f"a0A{_it}", bufs=1, space="PSUM") as psA:
                    def z0_store(hh):
                        def store(rb, oTt):
                            dst = (Zb0h[hh][2 * rb:2 * rb + 2, :, :] if cc_split4
                                   else Zb0[2 * rb:2 * rb + 2, hh, :, :])
                            nc.sync.dma_start(
                                dst.rearrange("j d t -> d j t"),
                                oTt[:, :, :].rearrange("p (j a) b -> p j (a b)",
                                                       j=2),
                            )
                        return store
                    # prefetch b1's first x block while attention runs
                    def pf_xb4():
                        xb4 = xp.tile([128, KT, 512], BF16, tag="xb", name="xb")
                        xTr = xT_e.rearrange("(kt p) t -> p kt t", p=128)
                        gsl = bass.ds(T, 512)
                        nc.sync.dma_start(xb4[:, :KT // 2, :], xTr[:, :KT // 2, gsl])
                        nc.sync.dma_start(xb4[:, KT // 2:, :], xTr[:, KT // 2:, gsl])
                        pf_xb4.tile = xb4
                    if SA >= 1:
                      attn_batch_head(0, 0, qk0, v0, tp2, tpo, tpr, ps2, psA,
                                      z0_store(0), between={1: pf_xb4})
                      if cc_split4:
                        if fake_cc:
                            nc.sync.dma_start(ZGb0h[0][:], Zb0h[0][:])
                        else:
                            nc.gpsimd.collective_compute(
                                "AllToAll", ALU.bypass,
                                replica_groups=[list(range(N_CORES))],
                                ins=[Zb0h[0][:]], outs=[ZGb0h[0][:]],
                            )
                      attn_batch_head(0, 1, qk0, v0, tp2, tpo, tpr, ps2, psA,
                                      z0_store(1))

            if SA < 1:
                pass
            elif cc_split4:
                if fake_cc:
                    nc.sync.dma_start(ZGb0h[1][:], Zb0h[1][:])
                else:
                    nc.gpsimd.collective_compute(
                        "AllToAll", ALU.bypass,
                        replica_groups=[list(range(N_CORES))],
                        ins=[Zb0h[1][:]], outs=[ZGb0h[1][:]],
                    )
            elif fake_cc:
                nc.sync.dma_start(ZGb0[:], Zb0[:])
            else:
                nc.gpsimd.collective_compute(
                    "AllToAll", ALU.bypass,
                    replica_groups=[list(range(N_CORES))],
                    ins=[Zb0[:]], outs=[ZGb0[:]],
                )
            # out_w preload (8MB) on the gpsimd queue; lands in SBUF freed by
            # the batch-0 pools, well before the out-projection needs it.
            with tc.tile_pool(name=f"ow{_it}", bufs=1) as owp:
                # ow contraction-tile order kt = h = 2*s + i (head index);
                # loaded in quarters on the sync queue, interleaved with the
                # b1 x-block prefetches (hardware DGE; the swdge path on
                # gpsimd is software-paced and slow on real hardware)
                ow = owp.tile([128, KT, D], BF16, tag="ow", name="ow")
                owr = owT_e.rearrange("(kt p) f -> p kt f", p=128)

                def ow_load(q):
                    nc.sync.dma_start(
                        ow[:, bass.ts(q, KT // 4), :], owr[:, bass.ts(q, KT // 4), :])

                # ================= batch 1 =================
                with tc.tile_pool(name=f"b1{_it}", bufs=1) as bp1, \
                     tc.tile_pool(name=f"b1t{_it}", bufs=3) as tp1:
                    qk1 = [bp1.tile([128, T], BF16, tag=f"qk{m}", name=f"b1qk{m}")
                           for m in range(4)]
                    v1 = bp1.tile([128, T // 128, HPC, HD + 1], BF16, tag="v",
                                  name="b1v")
                    with tc.tile_pool(name=f"b1ps{_it}", bufs=6, space="PSUM") as ps1:
                      for blk in range(BLK if SA >= 2 else 0):
                        if blk == 0:
                            # first block's x was prefetched during attn(b0)
                            qkv_block(1, 0, qk1, v1, tp1, ps1, xb=pf_xb4.tile)
                            ow_load(0)
                        else:
                            qkv_block(1, blk, qk1, v1, tp1, ps1)
                            ow_load(blk)

                    # zg_b0 pull on the (now idle) sync queue; waits on CC(b0)
                    for i in range(HPC if SA >= 2 else 0):
                        if cc_split4:
                            src0 = Zb0h[i] if cc_nodep else ZGb0h[i]
                            nc.sync.dma_start(
                                zg_b0[:, i, :, :],
                                src0.rearrange("s d t -> d s t"))
                        else:
                            src0 = Zb0 if cc_nodep else ZGb0
                            nc.sync.dma_start(
                                zg_b0[:, i, :, :],
                                src0[:, i, :, :].rearrange("s d t -> d s t"))

                    with tc.tile_pool(name=f"a1t{_it}", bufs=3) as tp2, \
                         tc.tile_pool(name=f"a1o2{_it}", bufs=6) as tpo, \
                         tc.tile_pool(name=f"a1r2{_it}", bufs=8) as tpr, \
                         tc.tile_pool(name=f"a1s{_it}", bufs=2, space="PSUM") as ps2, \
                         tc.tile_pool(name=f"a1A{_it}", bufs=1, space="PSUM") as psA:
                        def z1_store(hh):
                            def store(rb, oTt):
                                dst = (Zb1m[2 * rb:2 * rb + 2, hh, :, :]
                                       if cc_merge1 else
                                       Zb1[hh][2 * rb:2 * rb + 2, :, :])
                                nc.sync.dma_start(
                                    dst.rearrange("j d t -> d j t"),
                                    oTt[:, :, :].rearrange("p (j a) b -> p j (a b)",
                                                           j=2),
                                )
                            return store
                        for hh in range(HPC if SA >= 3 else 0):
                            attn_batch_head(1, hh, qk1, v1, tp2, tpo, tpr,
                                            ps2, psA, z1_store(hh))

                if SA < 3:
                    pass
                elif cc_merge1:
                    if fake_cc:
                        nc.sync.dma_start(ZGb1m[:], Zb1m[:])
                    else:
                        nc.gpsimd.collective_compute(
                            "AllToAll", ALU.bypass,
                            replica_groups=[list(range(N_CORES))],
                            ins=[Zb1m[:]], outs=[ZGb1m[:]],
                        )
                else:
                    for hh in range(HPC):
                        if fake_cc:
                            nc.sync.dma_start(ZGb1[hh][:], Zb1[hh][:])
                        else:
                            nc.gpsimd.collective_compute(
                                "AllToAll", ALU.bypass,
                                replica_groups=[list(range(N_CORES))],
                                ins=[Zb1[hh][:]], outs=[ZGb1[hh][:]],
                            )

                # ================= out projection =================
                with tc.tile_pool(name=f"p4t{_it}", bufs=4) as tp4, \
                     tc.tile_pool(name=f"p4ps{_it}", bufs=4, space="PSUM") as ps4:
                  if SA > 3:
                    # pass 1: batch-0 tokens, full 2048-feature contraction
                    for e in range(EB):
                        for tt in range(SB // 128):
                            pso4 = ps4.tile([128, 512], F32, tag="ps4", name="ps4")
                            for zt in range(KT):
                                nc.tensor.matmul(
                                    pso4[:],
                                    zg_b0[:, zt % 2, zt // 2, bass.ts(tt, 128)],
                                    ow[:, zt, bass.ts(e, 512)],
                                    start=(zt == 0),
                                    stop=(zt == KT - 1),
                                )
                            of = tp4.tile([128, 512], F32, tag="of", name="of")
                            nc.vector.tensor_add(of[:], pso4[:], obB[:, bass.ts(e, 512)])
                            nc.sync.dma_start(
                                out_e[bass.ds(tt * 128, 128), bass.ts(e, 512)], of[:])
                    # zg_b1 pulls on the scalar queue (idle after attention)
                    if cc_merge1:
                        src1a = (Zb1m if cc_nodep else ZGb1m)[:, 0, :, :]
                    else:
                        src1a = (Zb1[0] if cc_nodep else ZGb1[0])[:]
                    nc.scalar.dma_start(
                        zg_b1[:, 0, :, :], src1a.rearrange("s d t -> d s t"))
                    # pass 2a: batch-1 tokens, h0 (even-head) half of contraction
                    for e in range(EB):
                        for tt in range(SB // 128):
                            pso4 = ps4.tile([128, 512], F32, tag="ps4", name="ps4")
                            for s in range(N_CORES):
                                nc.tensor.matmul(
                                    pso4[:],
                                    zg_b1[:, 0, s, bass.ts(tt, 128)],
                                    ow[:, 2 * s, bass.ts(e, 512)],
                                    start=(s == 0),
                                    stop=(s == N_CORES - 1),
                                )
                            nc.vector.tensor_copy(
                                accbig[:, e * (SB // 128) + tt, :], pso4[:])
                    if cc_merge1:
                        src1b = (Zb1m if cc_nodep else ZGb1m)[:, 1, :, :]
                    else:
                        src1b = (Zb1[1] if cc_nodep else ZGb1[1])[:]
                    nc.scalar.dma_start(
                        zg_b1[:, 1, :, :], src1b.rearrange("s d t -> d s t"))
                    # pass 2b: h1 (odd-head) half + accumulated sum + bias
                    for e in range(EB):
                        for tt in range(SB // 128):
                            pso4 = ps4.tile([128, 512], F32, tag="ps4", name="ps4")
                            for s in range(N_CORES):
                                nc.tensor.matmul(
                                    pso4[:],
                                    zg_b1[:, 1, s, bass.ts(tt, 128)],
                                    ow[:, 2 * s + 1, bass.ts(e, 512)],
                                    start=(s == 0),
                                    stop=(s == N_CORES - 1),
                                )
                            mid = tp4.tile([128, 512], F32, tag="mid", name="mid")
                            nc.vector.tensor_add(
                                mid[:], pso4[:], accbig[:, e * (SB // 128) + tt, :])
                            of = tp4.tile([128, 512], F32, tag="of", name="of")
                            nc.vector.tensor_add(of[:], mid[:], obB[:, bass.ts(e, 512)])
                            nc.sync.dma_start(
                                out_e[bass.ds(SB + tt * 128, 128), bass.ts(e, 512)],
                                of[:])

    nc.compile()          # Bacc pass pipeline (library loads, nop fusion, regs)
    legalize_waits(nc)    # must run after all nop-fusion passes
    bass.Bass.finalize(nc)  # freeze without re-running Bacc compile
    return nc


def _prep_inputs(x, rope_cos, rope_sin, qkv_w, qkv_b, out_w, out_b, B, T, D, H):
    HD = D // H
    NT = B * T
    HPC = H // N_CORES
    bf = ml_dtypes.bfloat16

    x2 = np.ascontiguousarray(x.reshape(NT, D).T).astype(bf)           # [D, NT]
    cosT = np.ascontiguousarray(rope_cos[0, 0].T).astype(bf)           # [HD, T]
    s2 = rope_sin[0, 0].T.copy()
    s2[:HD // 2] *= -1.0
    sinT = np.ascontiguousarray(s2).astype(bf)
    owT = np.ascontiguousarray(out_w.T).astype(bf)                      # [D, D]
    ob = out_b.astype(np.float32)

    c_grid = np.arange(512)[None, :]
    p_grid = np.arange(128)[:, None]
    masks = np.stack(
        [(c_grid >= 128 * m + p_grid) for m in range(4)]
    ).astype(bf)                                                        # [4,128,512]

    in_maps = []
    for c in range(N_CORES):
        heads = [HPC * c + i for i in range(HPC)]
        q_rows = np.concatenate([qkv_w[h * HD:(h + 1) * HD] for h in heads])
        k_rows = np.concatenate([qkv_w[D + h * HD:D + (h + 1) * HD] for h in heads])
        v_rows = np.concatenate(
            [np.concatenate(
                [qkv_w[2 * D + h * HD:2 * D + (h + 1) * HD],
                 np.zeros((1, D), qkv_w.dtype)])
             for h in heads])
        wqk = np.ascontiguousarray(np.concatenate([q_rows, k_rows]).T).astype(bf)
        wv = np.ascontiguousarray(v_rows.T).astype(bf)
        bq = np.concatenate([qkv_b[h * HD:(h + 1) * HD] for h in heads])
        bk = np.concatenate([qkv_b[D + h * HD:D + (h + 1) * HD] for h in heads])
        bqk = np.concatenate([bq, bk]).astype(np.float32)
        bv = np.concatenate(
            [np.concatenate(
                [qkv_b[2 * D + h * HD:2 * D + (h + 1) * HD], np.ones(1)])
             for h in heads]
        ).astype(np.float32)
        in_maps.append({
            "xT": x2, "wqk": wqk, "bqk": bqk, "wv": wv, "bv": bv,
            "cosT": cosT, "sinT": sinT, "masks": masks,
            "owT": owT, "ob": ob,
        })
    return in_maps


_NC_CACHE = {}

# build options used by the public kernel() entry point (and test.py timing)
KERNEL_KW = {"ilv": True}


def kernel(x, rope_cos, rope_sin, qkv_w, qkv_b, out_w, out_b):
    B, T, D = x.shape
    H = 16
    NT = B * T
    SB = T // N_CORES
    key = (B, T, D, H)
    if key not in _NC_CACHE:
        _NC_CACHE[key] = build_nc(B, T, D, H, **KERNEL_KW)
    nc = _NC_CACHE[key]
    in_maps = _prep_inputs(
        np.asarray(x), np.asarray(rope_cos), np.asarray(rope_sin),
        np.asarray(qkv_w), np.asarray(qkv_b), np.asarray(out_w),
        np.asarray(out_b), B, T, D, H,
    )
    res = run_bass_kernel_spmd(nc, in_maps, core_ids=list(range(N_CORES)))
    out = np.empty((NT, D), np.float32)
    for c in range(N_CORES):
        r = res.results[c]["out"]                       # [2*SB, D]
        out[c * SB:(c + 1) * SB] = r[:SB]               # batch-0 token slice
        out[T + c * SB:T + (c + 1) * SB] = r[SB:]       # batch-1 token slice
    return out.reshape(B, T, D)

